# revision 12
# baseline (speedup 1.0000x reference)
"""Cross-attention kernel for 8 Trainium2 NeuronCores.

Sharding: data-parallel over batch (B=2) x tensor-parallel over heads
(16 heads -> 4 groups of 4 heads).  Core c handles batch c//4, head
group c%4.  All operands are cast to bf16 on the host (halves the
input-DMA volume that gates attention start; rel-err budget is 2e-2).

ACT (ScalarE exp, ~147us busy) is the hard roofline: every other
engine's work is scheduled into its shadow.

Per core, for its 4 heads:
    K^T = Wk_g^T y_b^T        [256, 2048]   (y streams first; K GEMM
                               c-pipelined with the y chunk DMAs)
    Q^T_ib = Wq_g^T x_ib^T    [256, 256]    (JIT per i-block)
    V   = y_b Wv_g            [2048, 256]   (JIT inside i-block 0)
    S^T_h = K_h Q_h^T / 8; P^T = exp(S^T)   (ScalarE, the bottleneck)
    O^T_h (+row sums via a ones-column in V) = [V_h|1]^T P^T
    partial = (O^T/rowsum)^T Wp_g           [2048, 1024]

Pipeline: chunk k = (ib, n); tensor-queue emission leads ACT by one
slot (S(k+1) emitted before AV(k-1)) so i-block boundaries never
stall the ACT queue; proj lags one i-block.  PSUM: 4 banks S
double-buffer + 2 banks O accumulators + 2 banks shared V/Q/proj
pool.
"""

import numpy as np

B = 2
N = 2048          # query sequence length
M = 2048          # key sequence length
DIM = 1024
HEAD_DIM = 64
SCALE = HEAD_DIM ** -0.5
NCORES = 8
GH = 4            # heads per core
J = GH * HEAD_DIM # 256 projected columns per core
KC = DIM // 128   # 8 contraction chunks
NT = M // 128     # 16 key tiles
IBS = 256         # i-block size
IB = N // IBS     # 8 i-blocks
TOT = IB * NT     # 128 chunks

_NC = None


def _build():
    from contextlib import ExitStack

    import concourse.bass as bass
    import concourse.tile as tile
    from concourse import bacc, mybir
    from concourse.bass import ts, ds

    f32 = mybir.dt.float32
    bf16 = mybir.dt.bfloat16
    Exp = mybir.ActivationFunctionType.Exp

    nc = bacc.Bacc("TRN2", target_bir_lowering=False, debug=False,
                   num_devices=NCORES)
    xT = nc.dram_tensor("xT", [DIM, N], bf16, kind="ExternalInput").ap()
    yT = nc.dram_tensor("yT", [DIM, M], bf16, kind="ExternalInput").ap()
    wq = nc.dram_tensor("wq", [DIM, J], bf16, kind="ExternalInput").ap()
    wk = nc.dram_tensor("wk", [DIM, J], bf16, kind="ExternalInput").ap()
    wv = nc.dram_tensor("wv", [DIM, J], bf16, kind="ExternalInput").ap()
    wp = nc.dram_tensor("wp", [J, DIM], bf16, kind="ExternalInput").ap()
    out = nc.dram_tensor("out", [N, DIM], bf16, kind="ExternalOutput").ap()

    with tile.TileContext(nc) as tc, ExitStack() as top:
        wpool = top.enter_context(tc.tile_pool(name="weights", bufs=1))
        wq_sb = wpool.tile([128, KC, J], bf16, name="wq_sb")
        wk_sb = wpool.tile([128, KC, J], bf16, name="wk_sb")
        wv_sb = wpool.tile([128, KC, J], bf16, name="wv_sb")
        wp_sb = wpool.tile([128, 2, DIM], bf16, name="wp_sb")
        scr = wpool.tile([1, 2], bf16, name="scr")

        big = top.enter_context(tc.tile_pool(name="big", bufs=1))
        xt = big.tile([128, KC, N], bf16, name="xt")
        yt = big.tile([128, KC, M], bf16, name="yt")
        KT = [big.tile([128, M], bf16, name=f"kt{t}") for t in range(2)]
        V_sb = big.tile([128, NT, GH, HEAD_DIM + 1], bf16, name="v_sb")
        qpool = top.enter_context(tc.tile_pool(name="qpool", bufs=2))
        # ones column for the row-sum trick; V evacuation overwrites 0:64
        nc.vector.memset(V_sb, 1.0)
        # preload the Exp table so the first real activation doesn't pay
        # the ~2.7us ACT_TABLE_LOAD on the critical path
        nc.vector.memset(scr, 0.0)
        nc.scalar.activation(scr, scr, Exp, bias=0.0, scale=1.0)

        # ---- input streams: y first (K gates the first exp), then x --
        # fine-grained chunks so the c-pipelined K GEMM starts ASAP
        wk_r = wk.rearrange("(c p) j -> p c j", p=128)
        wq_r = wq.rearrange("(c p) j -> p c j", p=128)
        qs = [nc.sync, nc.scalar]
        dq = 0

        def dma(dst, src):
            nonlocal dq
            qs[dq % 2].dma_start(dst, src)
            dq += 1

        for c in range(KC):
            dma(wk_sb[:, c, :], wk_r[:, c, :])
        for c in range(KC):
            for ic in range(4):
                dma(yt[:, c, ts(ic, 512)], yT[ts(c, 128), ts(ic, 512)])
            if c == 3:
                for cc in range(KC):
                    dma(xt[:, cc, 0:IBS], xT[ts(cc, 128), 0:IBS])
            if c == 5:
                for cc in range(KC):
                    dma(wq_sb[:, cc, :], wq_r[:, cc, :])
        dma(wv_sb, wv.rearrange("(c p) j -> p c j", p=128))
        dma(wp_sb, wp.rearrange("(t p) c -> p t c", p=128))
        for c in range(KC):
            dma(xt[:, c, IBS:DIM], xT[ts(c, 128), IBS:DIM])
        for c in range(KC):
            dma(xt[:, c, DIM:N], xT[ts(c, 128), DIM:N])

        # ---- K^T (8 psum banks, c-pipelined with the y DMA chunks) ---
        with tc.tile_pool(name="kpsum", bufs=1, space="PSUM") as kpsum:
            kps = [kpsum.tile([128, 512], f32, name=f"kps{t}")
                   for t in range(8)]
            for c in range(KC):
                for jt in range(2):
                    for ic in range(4):
                        nc.tensor.matmul(
                            kps[jt * 4 + ic],
                            wk_sb[:, c, ts(jt, 128)],
                            yt[:, c, ts(ic, 512)],
                            start=(c == 0), stop=(c == KC - 1))
            # evacuation order frees the banks the attention pools WAR
            # against first: auxp (banks 6,7 -> Q0), spsum buf0 (0,1)
            # and the ic0 halves of KT (banks 0,4) that S(0) reads
            for t in (6, 7, 0, 1, 4, 2, 3, 5):
                jt, ic = divmod(t, 4)
                nc.vector.tensor_copy(KT[jt][:, ts(ic, 512)], kps[t])

        # ---- attention: chunk pipeline over (ib, n) ------------------
        ppool = top.enter_context(tc.tile_pool(name="ppool", bufs=4))
        otpool = top.enter_context(tc.tile_pool(name="otpool", bufs=6))
        rpool = top.enter_context(tc.tile_pool(name="rpool", bufs=4))
        rbpool = top.enter_context(tc.tile_pool(name="rbpool", bufs=4))
        obpool = top.enter_context(tc.tile_pool(name="obpool", bufs=4))
        rdram = top.enter_context(tc.tile_pool(name="rdram", bufs=3,
                                               space="DRAM"))
        ot_tiles = {}
        q_tiles = {}
        q_psum = {}
        PIECE_SLOT = {3: 0, 5: 1, 13: 2, 15: 3}
        # Q-JIT c-chunk pairs spread over slots (jt, [c0,c1] per slot)
        Q_SLOT = {6: (0, 0), 7: (0, 2), 8: (0, 4), 9: (0, 6),
                  10: (1, 0), 11: (1, 2), 12: (1, 4), 14: (1, 6)}

        with tc.tile_pool(name="spsum", bufs=2, space="PSUM") as spsum, \
             tc.tile_pool(name="opsum", bufs=1, space="PSUM") as opsum, \
             tc.tile_pool(name="auxp", bufs=2, space="PSUM") as auxp:

            def emit_q_piece(ib, jt, c0):
                # two c-chunks of the Q^T accumulation per slot so the
                # PE burst stays inside the per-slot ACT slack
                if jt == 0 and c0 == 0:
                    q_tiles[ib] = qpool.tile([128, 2, IBS], bf16,
                                             name="qt")
                if c0 == 0:
                    q_psum[(ib, jt)] = auxp.tile([128, IBS], f32,
                                                 name="qp", tag="aux")
                qp = q_psum[(ib, jt)]
                i_sl = ts(ib, IBS)
                for c in (c0, c0 + 1):
                    nc.tensor.matmul(
                        qp,
                        wq_sb[:, c, ts(jt, 128)],
                        xt[:, c, i_sl],
                        start=(c == 0), stop=(c == KC - 1))
                if c0 + 2 == KC:
                    nc.vector.tensor_copy(q_tiles[ib][:, jt, :],
                                          q_psum.pop((ib, jt)))

            def emit_q_half(ib, jt):
                for c0 in range(0, KC, 2):
                    emit_q_piece(ib, jt, c0)

            def emit_v_tile(n):
                vp = auxp.tile([128, J], f32, name="vp", tag="aux")
                for c in range(KC):
                    nc.tensor.matmul(
                        vp,
                        yt[:, c, ts(n, 128)],
                        wv_sb[:, c, :],
                        start=(c == 0), stop=(c == KC - 1))
                nc.vector.tensor_copy(
                    V_sb[:, n, :, 0:HEAD_DIM],
                    vp.rearrange("p (h d) -> p h d", h=GH))

            def emit_proj_piece(ib, piece):
                icr, cc = divmod(piece, DIM // 512)
                op = auxp.tile([128, 512], f32, name="op", tag="aux")
                for pr in range(2):
                    nc.tensor.matmul(
                        op,
                        ot_tiles[(ib, pr)][:, ts(icr, 128)],
                        wp_sb[:, pr, ts(cc, 512)],
                        start=(pr == 0), stop=(pr == 1))
                ob = obpool.tile([128, 512], bf16, name="ob")
                nc.vector.tensor_copy(ob, op)
                nc.sync.dma_start(
                    out[ds(ib * IBS + icr * 128, 128), ts(cc, 512)],
                    ob)

            sps = {}
            pts = {}
            oaccs = {}

            def emit_s(k):
                ib, n = divmod(k, NT)
                qt = q_tiles[ib]
                sp = spsum.tile([128, 2, 2, IBS], f32, name="sp")
                for pr in range(2):
                    nc.tensor.matmul(
                        sp[:, 0, pr, :],
                        KT[pr][0:64, ts(n, 128)],
                        qt[0:64, pr, :],
                        start=True, stop=True, tile_position=(0, 0))
                    nc.tensor.matmul(
                        sp[:, 1, pr, :],
                        KT[pr][64:128, ts(n, 128)],
                        qt[64:128, pr, :],
                        start=True, stop=True, tile_position=(64, 0))
                sps[k] = sp

            def emit_exp(k):
                pt = ppool.tile([128, 2, 2, IBS], bf16, name="pt")
                nc.scalar.activation(pt, sps.pop(k), Exp, bias=0.0,
                                     scale=float(SCALE))
                pts[k] = pt

            def emit_av(k):
                ib, n = divmod(k, NT)
                if n == 0:
                    oaccs[ib] = [opsum.tile([HEAD_DIM + 1, 2, IBS], f32,
                                            name=f"oa{pr}")
                                 for pr in range(2)]
                oacc = oaccs[ib]
                pt = pts.pop(k)
                for pr in range(2):
                    for lh in range(2):
                        nc.tensor.matmul(
                            oacc[pr][:, lh, :],
                            V_sb[:, n, 2 * pr + lh, :],
                            pt[:, lh, pr, :],
                            start=(n == 0 and lh == 0),
                            stop=(n == NT - 1 and lh == 1))
                if n == NT - 1:
                    emit_norm(ib)

            def emit_norm(ib):
                # evacuate + normalize (reciprocal rowsum broadcast via
                # a DRAM round trip; batched store, and the last block
                # splits its two chains across the sync/scalar queues)
                oacc = oaccs.pop(ib)
                blk = {}
                for pr in range(2):
                    ot = otpool.tile([128, IBS], bf16, name="ot")
                    nc.vector.tensor_copy(ot[0:64, :], oacc[pr][0:64, 0, :])
                    nc.vector.tensor_copy(ot[64:128, :],
                                          oacc[pr][0:64, 1, :])
                    rs_lo = rpool.tile([1, IBS], f32, name="rslo")
                    rs_hi = rpool.tile([1, IBS], f32, name="rshi")
                    nc.vector.tensor_copy(rs_lo, oacc[pr][64:65, 0, :])
                    nc.vector.tensor_copy(rs_hi, oacc[pr][64:65, 1, :])
                    blk[pr] = (ot, rs_lo, rs_hi)
                for pr in range(2):
                    ot, rs_lo, rs_hi = blk[pr]
                    q = nc.scalar if (ib == IB - 1 and pr == 1) else nc.sync
                    rd = rdram.tile([2, IBS], f32, name="rd")
                    q.dma_start(rd[0:1, :], rs_lo)
                    q.dma_start(rd[1:2, :], rs_hi)
                    rb = rbpool.tile([128, IBS], f32, name="rb")
                    q.dma_start(rb[0:64, :],
                                rd[0:1, :].partition_broadcast(64))
                    q.dma_start(rb[64:128, :],
                                rd[1:2, :].partition_broadcast(64))
                    rb2 = rbpool.tile([128, IBS], f32, name="rb2")
                    nc.vector.reciprocal_approx_fast(rb2, rb)
                    nc.vector.tensor_mul(ot, ot, rb2)
                    ot_tiles[(ib, pr)] = ot

            emit_q_half(0, 0)
            emit_q_half(0, 1)
            emit_v_tile(0)
            for k in range(TOT + 2):
                if k < TOT:
                    emit_s(k)
                    ib, n = divmod(k, NT)
                    if ib == 0 and n + 1 < NT:
                        emit_v_tile(n + 1)
                    if n in Q_SLOT and ib + 1 < IB:
                        jt, c0 = Q_SLOT[n]
                        emit_q_piece(ib + 1, jt, c0)
                    if ib >= 1 and n in PIECE_SLOT:
                        emit_proj_piece(ib - 1, PIECE_SLOT[n])
                    emit_exp(k)
                if k >= 2:
                    emit_av(k - 2)
            for piece in range(4):
                emit_proj_piece(IB - 1, piece)

    nc.compile()
    return nc


def _get_nc():
    global _NC
    if _NC is None:
        _NC = _build()
    return _NC


def _shard_inputs(x, y, Wq, Wk, Wv, Wp):
    import ml_dtypes
    bf = ml_dtypes.bfloat16
    x = np.asarray(x, np.float32)
    y = np.asarray(y, np.float32)
    Wq = np.asarray(Wq, np.float32)
    Wk = np.asarray(Wk, np.float32)
    Wv = np.asarray(Wv, np.float32)
    Wp = np.asarray(Wp, np.float32)
    xT = [np.ascontiguousarray(x[b].T).astype(bf) for b in range(B)]
    yT = [np.ascontiguousarray(y[b].T).astype(bf) for b in range(B)]
    in_maps = []
    for c in range(NCORES):
        b, g = divmod(c, NCORES // B)
        sl = slice(g * J, (g + 1) * J)
        in_maps.append({
            "xT": xT[b],
            "yT": yT[b],
            "wq": np.ascontiguousarray(Wq[:, sl]).astype(bf),
            "wk": np.ascontiguousarray(Wk[:, sl]).astype(bf),
            "wv": np.ascontiguousarray(Wv[:, sl]).astype(bf),
            "wp": np.ascontiguousarray(Wp[sl, :]).astype(bf),
        })
    return in_maps


def run(inputs, trace=False, **spmd_kwargs):
    from concourse.bass_utils import run_bass_kernel_spmd
    nc = _get_nc()
    in_maps = _shard_inputs(inputs["x"], inputs["y"], inputs["Wq"],
                            inputs["Wk"], inputs["Wv"], inputs["Wp"])
    res = run_bass_kernel_spmd(nc, in_maps, core_ids=list(range(NCORES)),
                               trace=trace, **spmd_kwargs)
    bp = np.asarray(inputs["bp"], np.float32)
    gpb = NCORES // B
    full = np.empty((B, N, DIM), np.float32)
    for b in range(B):
        acc = res.results[b * gpb]["out"].astype(np.float32)
        for g in range(1, gpb):
            acc = acc + res.results[b * gpb + g]["out"].astype(np.float32)
        full[b] = acc + bp
    return full, res


def kernel(**inputs):
    out, _ = run(inputs, trace=False)
    return out


# revision 19
# speedup vs baseline: 1.0699x; 1.0699x over previous
"""Cross-attention kernel for 8 Trainium2 NeuronCores.

Sharding: data-parallel over batch (B=2) x tensor-parallel over heads
(16 heads -> 4 groups of 4 heads).  Core c handles batch c//4, head
group c%4.  All operands are cast to bf16 on the host (halves the
input-DMA volume that gates attention start; rel-err budget is 2e-2).

ACT (ScalarE exp, ~147us busy) is the hard roofline: every other
engine's work is scheduled into its shadow.

Per core, for its 4 heads:
    K^T = Wk_g^T y_b^T        [256, 2048]   (y streams first; K GEMM
                               c-pipelined with the y chunk DMAs)
    Q^T_ib = Wq_g^T x_ib^T    [256, 256]    (JIT per i-block)
    V   = y_b Wv_g            [2048, 256]   (JIT inside i-block 0)
    S^T_h = K_h Q_h^T / 8; P^T = exp(S^T)   (ScalarE, the bottleneck)
    O^T_h (+row sums via a ones-column in V) = [V_h|1]^T P^T
    partial = (O^T/rowsum)^T Wp_g           [2048, 1024]

Pipeline: chunk k = (ib, n); tensor-queue emission leads ACT by one
slot (S(k+1) emitted before AV(k-1)) so i-block boundaries never
stall the ACT queue; proj lags one i-block.  PSUM: 4 banks S
double-buffer + 2 banks O accumulators + 2 banks shared V/Q/proj
pool.
"""

import numpy as np

B = 2
N = 2048          # query sequence length
M = 2048          # key sequence length
DIM = 1024
HEAD_DIM = 64
SCALE = HEAD_DIM ** -0.5
NCORES = 8
GH = 4            # heads per core
J = GH * HEAD_DIM # 256 projected columns per core
KC = DIM // 128   # 8 contraction chunks
NT = M // 128     # 16 key tiles
IBS = 256         # i-block size
IB = N // IBS     # 8 i-blocks
TOT = IB * NT     # 128 chunks

_NC = None


def _build():
    from contextlib import ExitStack

    import concourse.bass as bass
    import concourse.tile as tile
    from concourse import bacc, mybir
    from concourse.bass import ts, ds

    f32 = mybir.dt.float32
    bf16 = mybir.dt.bfloat16
    Exp = mybir.ActivationFunctionType.Exp

    nc = bacc.Bacc("TRN2", target_bir_lowering=False, debug=False,
                   num_devices=NCORES)
    xT = nc.dram_tensor("xT", [DIM, N], bf16, kind="ExternalInput").ap()
    yT = nc.dram_tensor("yT", [DIM, M], bf16, kind="ExternalInput").ap()
    wq = nc.dram_tensor("wq", [DIM, J], bf16, kind="ExternalInput").ap()
    wk = nc.dram_tensor("wk", [DIM, J], bf16, kind="ExternalInput").ap()
    wv = nc.dram_tensor("wv", [DIM, J], bf16, kind="ExternalInput").ap()
    wp = nc.dram_tensor("wp", [J, DIM], bf16, kind="ExternalInput").ap()
    out = nc.dram_tensor("out", [N, DIM], bf16, kind="ExternalOutput").ap()

    with tile.TileContext(nc) as tc, ExitStack() as top:
        wpool = top.enter_context(tc.tile_pool(name="weights", bufs=1))
        wq_sb = wpool.tile([128, KC, J], bf16, name="wq_sb")
        wk_sb = wpool.tile([128, KC, J], bf16, name="wk_sb")
        wv_sb = wpool.tile([128, KC, J], bf16, name="wv_sb")
        wp_sb = wpool.tile([128, 2, DIM], bf16, name="wp_sb")
        scr = wpool.tile([1, 2], bf16, name="scr")

        big = top.enter_context(tc.tile_pool(name="big", bufs=1))
        xt = big.tile([128, KC, N], bf16, name="xt")
        yt = big.tile([128, KC, M], bf16, name="yt")
        KT = [big.tile([128, M], bf16, name=f"kt{t}") for t in range(2)]
        V_sb = big.tile([128, NT, GH, HEAD_DIM + 1], bf16, name="v_sb")
        warm = big.tile([128, 512], bf16, name="warm")
        qpool = top.enter_context(tc.tile_pool(name="qpool", bufs=2))
        # ones column for the row-sum trick; V evacuation overwrites 0:64
        nc.vector.memset(V_sb, 1.0)
        nc.vector.memset(warm, 0.0)
        # preload the Exp table so the first real activation doesn't pay
        # the ~2.7us ACT_TABLE_LOAD on the critical path
        nc.vector.memset(scr, 0.0)
        nc.scalar.activation(scr, scr, Exp, bias=0.0, scale=1.0)

        # ---- input streams: y first (K gates the first exp), then x --
        # scalar (ACT) queue gets only 4 cheap enqueues; bulky/late
        # tensors ride the idle GpSimd SWDGE queue
        wq_r = wq.rearrange("(c p) j -> p c j", p=128)
        for c in range(0, KC, 2):
            nc.scalar.dma_start(yt[:, c, :], yT[ts(c, 128), :])
        nc.sync.dma_start(wk_sb, wk.rearrange("(c p) j -> p c j", p=128))
        nc.sync.dma_start(yt[:, 1, :], yT[ts(1, 128), :])
        nc.sync.dma_start(xt[:, :, 0:IBS],
                          xT.rearrange("(c p) i -> p c i", p=128)[:, :, 0:IBS])
        nc.sync.dma_start(wq_sb, wq_r)
        nc.sync.dma_start(yt[:, 3, :], yT[ts(3, 128), :])
        nc.sync.dma_start(yt[:, 5, :], yT[ts(5, 128), :])
        nc.sync.dma_start(yt[:, 7, :], yT[ts(7, 128), :])
        nc.gpsimd.dma_start(wv_sb, wv.rearrange("(c p) j -> p c j", p=128))
        nc.gpsimd.dma_start(
            xt[:, :, IBS:DIM],
            xT.rearrange("(c p) i -> p c i", p=128)[:, :, IBS:DIM])
        nc.gpsimd.dma_start(wp_sb, wp.rearrange("(t p) c -> p t c", p=128))
        nc.gpsimd.dma_start(
            xt[:, :, DIM:N],
            xT.rearrange("(c p) i -> p c i", p=128)[:, :, DIM:N])

        # ---- PE warm-up: ~3.4us of dummy matmuls lifts the HAM clock
        # gate to 8/8 before the K GEMM arrives ------------------------
        with tc.tile_pool(name="warmp", bufs=2, space="PSUM") as warmp:
            wps = [warmp.tile([128, 512], f32, name=f"wps{t}")
                   for t in range(2)]
            for i in range(16):
                nc.tensor.matmul(wps[i % 2], warm[:, 0:128], warm,
                                 start=True, stop=True)

        # ---- K^T (8 psum banks, c-pipelined with the y DMA chunks) ---
        with tc.tile_pool(name="kpsum", bufs=1, space="PSUM") as kpsum:
            kps = [kpsum.tile([128, 512], f32, name=f"kps{t}")
                   for t in range(8)]
            for c in range(KC):
                for jt in range(2):
                    for ic in range(4):
                        nc.tensor.matmul(
                            kps[jt * 4 + ic],
                            wk_sb[:, c, ts(jt, 128)],
                            yt[:, c, ts(ic, 512)],
                            start=(c == 0), stop=(c == KC - 1))
            # evacuation order frees the banks the attention pools WAR
            # against first: auxp (banks 6,7 -> Q0), spsum buf0 (0,1)
            # and the ic0 halves of KT (banks 0,4) that S(0) reads
            for t in (6, 7, 0, 1, 4, 2, 3, 5):
                jt, ic = divmod(t, 4)
                nc.vector.tensor_copy(KT[jt][:, ts(ic, 512)], kps[t])

        # ---- attention: chunk pipeline over (ib, n) ------------------
        ppool = top.enter_context(tc.tile_pool(name="ppool", bufs=5))
        otpool = top.enter_context(tc.tile_pool(name="otpool", bufs=8))
        rpool = top.enter_context(tc.tile_pool(name="rpool", bufs=4))
        rbpool = top.enter_context(tc.tile_pool(name="rbpool", bufs=4))
        obpool = top.enter_context(tc.tile_pool(name="obpool", bufs=6))
        rdram = top.enter_context(tc.tile_pool(name="rdram", bufs=3,
                                               space="DRAM"))
        ot_tiles = {}
        q_tiles = {}
        q_psum = {}
        # Q-JIT c-chunk pairs spread over slots (jt, c0 per slot)
        Q_SLOT = {5: (0, 0), 6: (0, 2), 7: (0, 4), 8: (0, 6),
                  9: (1, 0), 10: (1, 2), 11: (1, 4), 12: (1, 6)}

        with tc.tile_pool(name="spsum", bufs=2, space="PSUM") as spsum, \
             tc.tile_pool(name="opsum", bufs=1, space="PSUM") as opsum, \
             tc.tile_pool(name="auxp", bufs=2, space="PSUM") as auxp:

            def emit_q_piece(ib, jt, c0):
                # two c-chunks of the Q^T accumulation per slot so the
                # PE burst stays inside the per-slot ACT slack
                if jt == 0 and c0 == 0:
                    q_tiles[ib] = qpool.tile([128, 2, IBS], bf16,
                                             name="qt")
                if c0 == 0:
                    q_psum[(ib, jt)] = auxp.tile([128, IBS], f32,
                                                 name="qp", tag="aux")
                qp = q_psum[(ib, jt)]
                i_sl = ts(ib, IBS)
                for c in (c0, c0 + 1):
                    nc.tensor.matmul(
                        qp,
                        wq_sb[:, c, ts(jt, 128)],
                        xt[:, c, i_sl],
                        start=(c == 0), stop=(c == KC - 1))
                if c0 + 2 == KC:
                    nc.vector.tensor_copy(q_tiles[ib][:, jt, :],
                                          q_psum.pop((ib, jt)))

            def emit_q_half(ib, jt):
                for c0 in range(0, KC, 2):
                    emit_q_piece(ib, jt, c0)

            def emit_v_tile(n):
                vp = auxp.tile([128, J], f32, name="vp", tag="aux")
                for c in range(KC):
                    nc.tensor.matmul(
                        vp,
                        yt[:, c, ts(n, 128)],
                        wv_sb[:, c, :],
                        start=(c == 0), stop=(c == KC - 1))
                nc.vector.tensor_copy(
                    V_sb[:, n, :, 0:HEAD_DIM],
                    vp.rearrange("p (h d) -> p h d", h=GH))

            def emit_proj_piece(ib, piece):
                icr, cc = divmod(piece, DIM // 512)
                op = auxp.tile([128, 512], f32, name="op", tag="aux")
                for pr in range(2):
                    nc.tensor.matmul(
                        op,
                        ot_tiles[(ib, pr)][:, ts(icr, 128)],
                        wp_sb[:, pr, ts(cc, 512)],
                        start=(pr == 0), stop=(pr == 1))
                ob = obpool.tile([128, 512], bf16, name="ob")
                nc.vector.tensor_copy(ob, op)
                nc.sync.dma_start(
                    out[ds(ib * IBS + icr * 128, 128), ts(cc, 512)],
                    ob)

            sps = {}
            pts = {}
            oaccs = {}

            def emit_s(k):
                ib, n = divmod(k, NT)
                qt = q_tiles[ib]
                sp = spsum.tile([128, 2, 2, IBS], f32, name="sp")
                for pr in range(2):
                    nc.tensor.matmul(
                        sp[:, 0, pr, :],
                        KT[pr][0:64, ts(n, 128)],
                        qt[0:64, pr, :],
                        start=True, stop=True, tile_position=(0, 0))
                    nc.tensor.matmul(
                        sp[:, 1, pr, :],
                        KT[pr][64:128, ts(n, 128)],
                        qt[64:128, pr, :],
                        start=True, stop=True, tile_position=(64, 0))
                sps[k] = sp

            def emit_exp(k):
                pt = ppool.tile([128, 2, 2, IBS], bf16, name="pt")
                nc.scalar.activation(pt, sps.pop(k), Exp, bias=0.0,
                                     scale=float(SCALE))
                pts[k] = pt

            def emit_av(k):
                ib, n = divmod(k, NT)
                if n == 0:
                    oaccs[ib] = [opsum.tile([HEAD_DIM + 1, 2, IBS], f32,
                                            name=f"oa{pr}")
                                 for pr in range(2)]
                oacc = oaccs[ib]
                pt = pts.pop(k)
                for pr in range(2):
                    for lh in range(2):
                        nc.tensor.matmul(
                            oacc[pr][:, lh, :],
                            V_sb[:, n, 2 * pr + lh, :],
                            pt[:, lh, pr, :],
                            start=(n == 0 and lh == 0),
                            stop=(n == NT - 1 and lh == 1))
                if n == NT - 1:
                    emit_norm(ib)

            def emit_norm(ib):
                # evacuate + normalize (reciprocal rowsum broadcast via
                # a DRAM round trip; batched store, and the last block
                # splits its two chains across the sync/scalar queues)
                oacc = oaccs.pop(ib)
                blk = {}
                for pr in range(2):
                    ot = otpool.tile([128, IBS], bf16, name="ot")
                    nc.vector.tensor_copy(ot[0:64, :], oacc[pr][0:64, 0, :])
                    nc.vector.tensor_copy(ot[64:128, :],
                                          oacc[pr][0:64, 1, :])
                    rs_lo = rpool.tile([1, IBS], f32, name="rslo")
                    rs_hi = rpool.tile([1, IBS], f32, name="rshi")
                    nc.vector.tensor_copy(rs_lo, oacc[pr][64:65, 0, :])
                    nc.vector.tensor_copy(rs_hi, oacc[pr][64:65, 1, :])
                    blk[pr] = (ot, rs_lo, rs_hi)
                for pr in range(2):
                    ot, rs_lo, rs_hi = blk[pr]
                    q = nc.scalar if (ib == IB - 1 and pr == 1) else nc.sync
                    rd = rdram.tile([2, IBS], f32, name="rd")
                    q.dma_start(rd[0:1, :], rs_lo)
                    q.dma_start(rd[1:2, :], rs_hi)
                    rb = rbpool.tile([128, IBS], f32, name="rb")
                    q.dma_start(rb[0:64, :],
                                rd[0:1, :].partition_broadcast(64))
                    q.dma_start(rb[64:128, :],
                                rd[1:2, :].partition_broadcast(64))
                    rb2 = rbpool.tile([128, IBS], f32, name="rb2")
                    nc.vector.reciprocal_approx_fast(rb2, rb)
                    nc.vector.tensor_mul(ot, ot, rb2)
                    ot_tiles[(ib, pr)] = ot

            emit_q_half(0, 0)
            emit_q_half(0, 1)
            for k in range(TOT + 3):
                if k < TOT:
                    emit_s(k)
                    ib, n = divmod(k, NT)
                    if ib == 0:
                        emit_v_tile(n)
                    if n in Q_SLOT and ib + 1 < IB:
                        jt, c0 = Q_SLOT[n]
                        emit_q_piece(ib + 1, jt, c0)
                    # proj(ib-1) pieces 0,1 late in ib; pieces 2,3 early
                    # in ib+1 (the rowsum round trip isn't done before
                    # slot ~6); the last block takes the leftovers in
                    # its Q-free slots
                    if n == 1 and ib >= 2:
                        emit_proj_piece(ib - 2, 2)
                    if n == 3 and ib >= 2:
                        emit_proj_piece(ib - 2, 3)
                    if n == 13 and ib >= 1:
                        emit_proj_piece(ib - 1, 0)
                    if n == 15 and ib >= 1:
                        emit_proj_piece(ib - 1, 1)
                    if ib == IB - 1 and n == 7:
                        emit_proj_piece(IB - 2, 2)
                    if ib == IB - 1 and n == 9:
                        emit_proj_piece(IB - 2, 3)
                    emit_exp(k)
                if k >= 3:
                    emit_av(k - 3)
            for piece in range(4):
                emit_proj_piece(IB - 1, piece)

    nc.compile()
    return nc


def _get_nc():
    global _NC
    if _NC is None:
        _NC = _build()
    return _NC


def _shard_inputs(x, y, Wq, Wk, Wv, Wp):
    import ml_dtypes
    bf = ml_dtypes.bfloat16
    x = np.asarray(x, np.float32)
    y = np.asarray(y, np.float32)
    Wq = np.asarray(Wq, np.float32)
    Wk = np.asarray(Wk, np.float32)
    Wv = np.asarray(Wv, np.float32)
    Wp = np.asarray(Wp, np.float32)
    xT = [np.ascontiguousarray(x[b].T).astype(bf) for b in range(B)]
    yT = [np.ascontiguousarray(y[b].T).astype(bf) for b in range(B)]
    in_maps = []
    for c in range(NCORES):
        b, g = divmod(c, NCORES // B)
        sl = slice(g * J, (g + 1) * J)
        in_maps.append({
            "xT": xT[b],
            "yT": yT[b],
            "wq": np.ascontiguousarray(Wq[:, sl]).astype(bf),
            "wk": np.ascontiguousarray(Wk[:, sl]).astype(bf),
            "wv": np.ascontiguousarray(Wv[:, sl]).astype(bf),
            "wp": np.ascontiguousarray(Wp[sl, :]).astype(bf),
        })
    return in_maps


def run(inputs, trace=False, **spmd_kwargs):
    from concourse.bass_utils import run_bass_kernel_spmd
    nc = _get_nc()
    in_maps = _shard_inputs(inputs["x"], inputs["y"], inputs["Wq"],
                            inputs["Wk"], inputs["Wv"], inputs["Wp"])
    res = run_bass_kernel_spmd(nc, in_maps, core_ids=list(range(NCORES)),
                               trace=trace, **spmd_kwargs)
    bp = np.asarray(inputs["bp"], np.float32)
    gpb = NCORES // B
    full = np.empty((B, N, DIM), np.float32)
    for b in range(B):
        acc = res.results[b * gpb]["out"].astype(np.float32)
        for g in range(1, gpb):
            acc = acc + res.results[b * gpb + g]["out"].astype(np.float32)
        full[b] = acc + bp
    return full, res


def kernel(**inputs):
    out, _ = run(inputs, trace=False)
    return out


# revision 23
# speedup vs baseline: 1.1402x; 1.0657x over previous
"""Cross-attention kernel for 8 Trainium2 NeuronCores.

Sharding: data-parallel over batch (B=2) x tensor-parallel over heads
(16 heads -> 4 groups of 4 heads).  Core c handles batch c//4, head
group c%4.  All operands are cast to bf16 on the host (halves the
input-DMA volume that gates attention start; rel-err budget is 2e-2).

ACT (ScalarE exp, ~147us busy) is the hard roofline: every other
engine's work is scheduled into its shadow.

Per core, for its 4 heads:
    K^T = Wk_g^T y_b^T        [256, 2048]   (y streams first; K GEMM
                               c-pipelined with the y chunk DMAs)
    Q^T_ib = Wq_g^T x_ib^T    [256, 256]    (JIT per i-block)
    V   = y_b Wv_g            [2048, 256]   (JIT inside i-block 0)
    S^T_h = K_h Q_h^T / 8; P^T = exp(S^T)   (ScalarE, the bottleneck)
    O^T_h (+row sums via a ones-column in V) = [V_h|1]^T P^T
    partial = (O^T/rowsum)^T Wp_g           [2048, 1024]

Pipeline: chunk k = (ib, n); tensor-queue emission leads ACT by one
slot (S(k+1) emitted before AV(k-1)) so i-block boundaries never
stall the ACT queue; proj lags one i-block.  PSUM: 4 banks S
double-buffer + 2 banks O accumulators + 2 banks shared V/Q/proj
pool.
"""

import numpy as np

B = 2
N = 2048          # query sequence length
M = 2048          # key sequence length
DIM = 1024
HEAD_DIM = 64
SCALE = HEAD_DIM ** -0.5
NCORES = 8
GH = 4            # heads per core
J = GH * HEAD_DIM # 256 projected columns per core
KC = DIM // 128   # 8 contraction chunks
NT = M // 128     # 16 key tiles
IBS = 256         # i-block size
IB = N // IBS     # 8 i-blocks
TOT = IB * NT     # 128 chunks

_NC = None


def _build():
    from contextlib import ExitStack

    import concourse.bass as bass
    import concourse.tile as tile
    from concourse import bacc, mybir
    from concourse.bass import ts, ds

    f32 = mybir.dt.float32
    bf16 = mybir.dt.bfloat16
    Exp = mybir.ActivationFunctionType.Exp

    nc = bacc.Bacc("TRN2", target_bir_lowering=False, debug=False,
                   num_devices=NCORES)
    xT = nc.dram_tensor("xT", [DIM, N], bf16, kind="ExternalInput").ap()
    yT = nc.dram_tensor("yT", [DIM, M], bf16, kind="ExternalInput").ap()
    wq = nc.dram_tensor("wq", [DIM, J], bf16, kind="ExternalInput").ap()
    wk = nc.dram_tensor("wk", [DIM, J], bf16, kind="ExternalInput").ap()
    wv = nc.dram_tensor("wv", [DIM, J], bf16, kind="ExternalInput").ap()
    wp = nc.dram_tensor("wp", [J, DIM], bf16, kind="ExternalInput").ap()
    out = nc.dram_tensor("out", [N, DIM], bf16, kind="ExternalOutput").ap()

    with tile.TileContext(nc) as tc, ExitStack() as top:
        wpool = top.enter_context(tc.tile_pool(name="weights", bufs=1))
        wq_sb = wpool.tile([128, KC, J], bf16, name="wq_sb")
        wk_sb = wpool.tile([128, KC, J], bf16, name="wk_sb")
        wv_sb = wpool.tile([128, KC, J], bf16, name="wv_sb")
        wp_sb = wpool.tile([128, 2, DIM], bf16, name="wp_sb")
        scr = wpool.tile([1, 2], bf16, name="scr")

        big = top.enter_context(tc.tile_pool(name="big", bufs=1))
        xt = big.tile([128, KC, N], bf16, name="xt")
        yt = big.tile([128, KC, M], bf16, name="yt")
        KT = [big.tile([128, M], bf16, name=f"kt{t}") for t in range(2)]
        V_sb = big.tile([128, NT, GH, HEAD_DIM + 1], bf16, name="v_sb")
        warm = big.tile([128, 512], bf16, name="warm")
        qpool = top.enter_context(tc.tile_pool(name="qpool", bufs=2))
        # ones column for the row-sum trick; V evacuation overwrites 0:64
        nc.vector.memset(V_sb, 1.0)
        nc.vector.memset(warm, 0.0)
        # preload the Exp table so the first real activation doesn't pay
        # the ~2.7us ACT_TABLE_LOAD on the critical path
        nc.vector.memset(scr, 0.0)
        nc.scalar.activation(scr, scr, Exp, bias=0.0, scale=1.0)

        # ---- input streams: y first (K gates the first exp), then x --
        # y split across all three DMA-capable queues (each sustains
        # only ~150-200 GB/s with serial chunk transfers); the scalar
        # (ACT) queue gets just 3 cheap enqueues
        x_r = xT.rearrange("(c p) i -> p c i", p=128)
        nc.scalar.dma_start(yt[:, 0, :], yT[ts(0, 128), :])
        nc.scalar.dma_start(yt[:, 2, :], yT[ts(2, 128), :])
        nc.scalar.dma_start(yt[:, 4, :], yT[ts(4, 128), :])
        nc.gpsimd.dma_start(wk_sb, wk.rearrange("(c p) j -> p c j", p=128))
        nc.sync.dma_start(yt[:, 1, :], yT[ts(1, 128), :])
        nc.sync.dma_start(yt[:, 3, :], yT[ts(3, 128), :])
        nc.sync.dma_start(xt[:, :, 0:IBS], x_r[:, :, 0:IBS])
        nc.sync.dma_start(wq_sb, wq.rearrange("(c p) j -> p c j", p=128))
        nc.gpsimd.dma_start(yt[:, 5, :], yT[ts(5, 128), :])
        nc.gpsimd.dma_start(yt[:, 6, :], yT[ts(6, 128), :])
        nc.gpsimd.dma_start(yt[:, 7, :], yT[ts(7, 128), :])
        nc.gpsimd.dma_start(wv_sb, wv.rearrange("(c p) j -> p c j", p=128))
        nc.gpsimd.dma_start(xt[:, :, IBS:DIM], x_r[:, :, IBS:DIM])
        nc.gpsimd.dma_start(wp_sb, wp.rearrange("(t p) c -> p t c", p=128))
        nc.gpsimd.dma_start(xt[:, :, DIM:N], x_r[:, :, DIM:N])

        # ---- PE warm-up: ~3.4us of dummy matmuls lifts the HAM clock
        # gate to 8/8 before the K GEMM arrives ------------------------
        with tc.tile_pool(name="warmp", bufs=2, space="PSUM") as warmp:
            wps = [warmp.tile([128, 512], f32, name=f"wps{t}")
                   for t in range(2)]
            for i in range(16):
                nc.tensor.matmul(wps[i % 2], warm[:, 0:128], warm,
                                 start=True, stop=True)

        # ---- K^T (8 psum banks, c-pipelined with the y DMA chunks) ---
        with tc.tile_pool(name="kpsum", bufs=1, space="PSUM") as kpsum:
            kps = [kpsum.tile([128, 512], f32, name=f"kps{t}")
                   for t in range(8)]
            for c in range(KC):
                for jt in range(2):
                    for ic in range(4):
                        nc.tensor.matmul(
                            kps[jt * 4 + ic],
                            wk_sb[:, c, ts(jt, 128)],
                            yt[:, c, ts(ic, 512)],
                            start=(c == 0), stop=(c == KC - 1))
            # evacuation order frees the banks the attention pools WAR
            # against first: auxp (banks 6,7 -> Q0), spsum buf0 (0,1)
            # and the ic0 halves of KT (banks 0,4) that S(0) reads
            for t in (6, 7, 0, 1, 4, 2, 3, 5):
                jt, ic = divmod(t, 4)
                nc.vector.tensor_copy(KT[jt][:, ts(ic, 512)], kps[t])

        # ---- attention: chunk pipeline over (ib, n) ------------------
        ppool = top.enter_context(tc.tile_pool(name="ppool", bufs=5))
        otpool = top.enter_context(tc.tile_pool(name="otpool", bufs=8))
        rpool = top.enter_context(tc.tile_pool(name="rpool", bufs=4))
        rbpool = top.enter_context(tc.tile_pool(name="rbpool", bufs=4))
        obpool = top.enter_context(tc.tile_pool(name="obpool", bufs=6))
        ot_tiles = {}
        q_tiles = {}
        q_psum = {}
        # Q-JIT c-chunk pairs spread over slots (jt, c0 per slot)
        Q_SLOT = {5: (0, 0), 6: (0, 2), 7: (0, 4), 8: (0, 6),
                  9: (1, 0), 10: (1, 2), 11: (1, 4), 12: (1, 6)}

        with tc.tile_pool(name="spsum", bufs=2, space="PSUM") as spsum, \
             tc.tile_pool(name="opsum", bufs=1, space="PSUM") as opsum, \
             tc.tile_pool(name="auxp", bufs=2, space="PSUM") as auxp:

            def emit_q_piece(ib, jt, c0):
                # two c-chunks of the Q^T accumulation per slot so the
                # PE burst stays inside the per-slot ACT slack
                if jt == 0 and c0 == 0:
                    q_tiles[ib] = qpool.tile([128, 2, IBS], bf16,
                                             name="qt")
                if c0 == 0:
                    q_psum[(ib, jt)] = auxp.tile([128, IBS], f32,
                                                 name="qp", tag="aux")
                qp = q_psum[(ib, jt)]
                i_sl = ts(ib, IBS)
                for c in (c0, c0 + 1):
                    nc.tensor.matmul(
                        qp,
                        wq_sb[:, c, ts(jt, 128)],
                        xt[:, c, i_sl],
                        start=(c == 0), stop=(c == KC - 1))
                if c0 + 2 == KC:
                    nc.vector.tensor_copy(q_tiles[ib][:, jt, :],
                                          q_psum.pop((ib, jt)))

            def emit_q_half(ib, jt):
                for c0 in range(0, KC, 2):
                    emit_q_piece(ib, jt, c0)

            def emit_v_tile(n):
                vp = auxp.tile([128, J], f32, name="vp", tag="aux")
                for c in range(KC):
                    nc.tensor.matmul(
                        vp,
                        yt[:, c, ts(n, 128)],
                        wv_sb[:, c, :],
                        start=(c == 0), stop=(c == KC - 1))
                nc.vector.tensor_copy(
                    V_sb[:, n, :, 0:HEAD_DIM],
                    vp.rearrange("p (h d) -> p h d", h=GH))

            def emit_proj_piece(ib, piece):
                icr, cc = divmod(piece, DIM // 512)
                op = auxp.tile([128, 512], f32, name="op", tag="aux")
                for pr in range(2):
                    nc.tensor.matmul(
                        op,
                        ot_tiles[(ib, pr)][:, ts(icr, 128)],
                        wp_sb[:, pr, ts(cc, 512)],
                        start=(pr == 0), stop=(pr == 1))
                ob = obpool.tile([128, 512], bf16, name="ob")
                nc.vector.tensor_copy(ob, op)
                nc.sync.dma_start(
                    out[ds(ib * IBS + icr * 128, 128), ts(cc, 512)],
                    ob)

            sps = {}
            pts = {}
            oaccs = {}

            def emit_s(k):
                ib, n = divmod(k, NT)
                qt = q_tiles[ib]
                sp = spsum.tile([128, 2, 2, IBS], f32, name="sp")
                for pr in range(2):
                    nc.tensor.matmul(
                        sp[:, 0, pr, :],
                        KT[pr][0:64, ts(n, 128)],
                        qt[0:64, pr, :],
                        start=True, stop=True, tile_position=(0, 0))
                    nc.tensor.matmul(
                        sp[:, 1, pr, :],
                        KT[pr][64:128, ts(n, 128)],
                        qt[64:128, pr, :],
                        start=True, stop=True, tile_position=(64, 0))
                sps[k] = sp

            def emit_exp(k):
                pt = ppool.tile([128, 2, 2, IBS], bf16, name="pt")
                nc.scalar.activation(pt, sps.pop(k), Exp, bias=0.0,
                                     scale=float(SCALE))
                pts[k] = pt

            def emit_av(k):
                ib, n = divmod(k, NT)
                if n == 0:
                    oaccs[ib] = [opsum.tile([HEAD_DIM + 1, 2, IBS], f32,
                                            name=f"oa{pr}")
                                 for pr in range(2)]
                oacc = oaccs[ib]
                pt = pts.pop(k)
                for pr in range(2):
                    for lh in range(2):
                        nc.tensor.matmul(
                            oacc[pr][:, lh, :],
                            V_sb[:, n, 2 * pr + lh, :],
                            pt[:, lh, pr, :],
                            start=(n == 0 and lh == 0),
                            stop=(n == NT - 1 and lh == 1))
                if n == NT - 1:
                    emit_norm(ib)

            def emit_norm(ib):
                # evacuate + normalize (reciprocal rowsum broadcast via
                # a DRAM round trip; batched store, and the last block
                # splits its two chains across the sync/scalar queues)
                oacc = oaccs.pop(ib)
                blk = {}
                for pr in range(2):
                    ot = otpool.tile([128, IBS], bf16, name="ot")
                    nc.vector.tensor_copy(ot[0:64, :], oacc[pr][0:64, 0, :])
                    nc.vector.tensor_copy(ot[64:128, :],
                                          oacc[pr][0:64, 1, :])
                    rs_lo = rpool.tile([1, IBS], f32, name="rslo")
                    rs_hi = rpool.tile([1, IBS], f32, name="rshi")
                    nc.vector.tensor_copy(rs_lo, oacc[pr][64:65, 0, :])
                    nc.vector.tensor_copy(rs_hi, oacc[pr][64:65, 1, :])
                    blk[pr] = (ot, rs_lo, rs_hi)
                for pr in range(2):
                    ot, rs_lo, rs_hi = blk[pr]
                    rb = rbpool.tile([128, IBS], f32, name="rb")
                    tmp = rpool.tile([64, IBS], f32, name="tmp")
                    # GpSimd's partition_broadcast only writes from
                    # partition 0, so the hi half goes via a tmp tile
                    # plus a cross-partition GpSimd copy
                    nc.gpsimd.partition_broadcast(rb[0:64, :], rs_lo,
                                                  channels=64)
                    nc.gpsimd.partition_broadcast(tmp, rs_hi, channels=64)
                    nc.gpsimd.tensor_copy(rb[64:128, :], tmp)
                    rb2 = rbpool.tile([128, IBS], f32, name="rb2")
                    nc.vector.reciprocal_approx_fast(rb2, rb)
                    nc.vector.tensor_mul(ot, ot, rb2)
                    ot_tiles[(ib, pr)] = ot

            emit_q_half(0, 0)
            emit_q_half(0, 1)
            for k in range(TOT + 3):
                if k < TOT:
                    emit_s(k)
                    ib, n = divmod(k, NT)
                    if ib == 0:
                        emit_v_tile(n)
                    if n in Q_SLOT and ib + 1 < IB:
                        jt, c0 = Q_SLOT[n]
                        emit_q_piece(ib + 1, jt, c0)
                    # proj(ib-1) pieces 0,1 late in ib; pieces 2,3 early
                    # in ib+1 (the rowsum round trip isn't done before
                    # slot ~6); the last block takes the leftovers in
                    # its Q-free slots
                    if n == 1 and ib >= 2:
                        emit_proj_piece(ib - 2, 2)
                    if n == 3 and ib >= 2:
                        emit_proj_piece(ib - 2, 3)
                    if n == 13 and ib >= 1:
                        emit_proj_piece(ib - 1, 0)
                    if n == 15 and ib >= 1:
                        emit_proj_piece(ib - 1, 1)
                    if ib == IB - 1 and n == 7:
                        emit_proj_piece(IB - 2, 2)
                    if ib == IB - 1 and n == 9:
                        emit_proj_piece(IB - 2, 3)
                    emit_exp(k)
                if k >= 3:
                    emit_av(k - 3)
            # keep the HAM clock warm while the last block's rowsum
            # broadcast chain runs, so the final proj isn't at 1.2 GHz
            wp_ps = auxp.tile([128, 512], f32, name="wtail", tag="aux")
            for i in range(8):
                nc.tensor.matmul(wp_ps, warm[:, 0:128], warm,
                                 start=True, stop=True)
            for piece in range(4):
                emit_proj_piece(IB - 1, piece)

    nc.compile()
    return nc


def _get_nc():
    global _NC
    if _NC is None:
        _NC = _build()
    return _NC


def _shard_inputs(x, y, Wq, Wk, Wv, Wp):
    import ml_dtypes
    bf = ml_dtypes.bfloat16
    x = np.asarray(x, np.float32)
    y = np.asarray(y, np.float32)
    Wq = np.asarray(Wq, np.float32)
    Wk = np.asarray(Wk, np.float32)
    Wv = np.asarray(Wv, np.float32)
    Wp = np.asarray(Wp, np.float32)
    xT = [np.ascontiguousarray(x[b].T).astype(bf) for b in range(B)]
    yT = [np.ascontiguousarray(y[b].T).astype(bf) for b in range(B)]
    in_maps = []
    for c in range(NCORES):
        b, g = divmod(c, NCORES // B)
        sl = slice(g * J, (g + 1) * J)
        in_maps.append({
            "xT": xT[b],
            "yT": yT[b],
            "wq": np.ascontiguousarray(Wq[:, sl]).astype(bf),
            "wk": np.ascontiguousarray(Wk[:, sl]).astype(bf),
            "wv": np.ascontiguousarray(Wv[:, sl]).astype(bf),
            "wp": np.ascontiguousarray(Wp[sl, :]).astype(bf),
        })
    return in_maps


def run(inputs, trace=False, **spmd_kwargs):
    from concourse.bass_utils import run_bass_kernel_spmd
    nc = _get_nc()
    in_maps = _shard_inputs(inputs["x"], inputs["y"], inputs["Wq"],
                            inputs["Wk"], inputs["Wv"], inputs["Wp"])
    res = run_bass_kernel_spmd(nc, in_maps, core_ids=list(range(NCORES)),
                               trace=trace, **spmd_kwargs)
    bp = np.asarray(inputs["bp"], np.float32)
    gpb = NCORES // B
    full = np.empty((B, N, DIM), np.float32)
    for b in range(B):
        acc = res.results[b * gpb]["out"].astype(np.float32)
        for g in range(1, gpb):
            acc = acc + res.results[b * gpb + g]["out"].astype(np.float32)
        full[b] = acc + bp
    return full, res


def kernel(**inputs):
    out, _ = run(inputs, trace=False)
    return out
